# revision 8
# baseline (speedup 1.0000x reference)
"""Tacotron2-style decoder on 8 Trainium2 NeuronCores.

Sharding strategy:
- Both LSTM cells sharded 8-way over the hidden dim (each core owns 128 of 1024
  hidden units = 512 of 4096 gate rows per weight matrix). Weights stay
  SBUF-resident in bf16 (~4.5 MB/core) instead of streaming ~71 MB/step.
- Location-sensitive attention sharded over batch (2 of 16 per core).
- Two AllGathers per step via DRAM bounce buffers: (ah(t), dh(t-1)) combined,
  and ctx(t). Activations are kept feature-on-partition ("transposed") so LSTM
  elementwise runs 128 lanes wide and gather loads are contiguous-run DMAs.

kernel(**inputs) takes full unsharded inputs (as reference.setup_inputs) and
returns (mel_outputs [16,80,64], gate_outputs [16,64], alignments [16,64,256]).
"""
import numpy as np
import ml_dtypes

import concourse.bass as bass
import concourse.bacc as bacc
import concourse.tile as tile
from concourse import mybir
from concourse import bass_utils

NCORES = 8
B, T_ENC, T_DEC = 16, 256, 64
N_MEL, E, H = 80, 512, 1024
A = 128
LOCK = 31
BLOC = B // NCORES  # 2 local batches per core
HS = H // NCORES    # 128 hidden units per core

BF16 = mybir.dt.bfloat16
F32 = mybir.dt.float32
AF = mybir.ActivationFunctionType
ALU = mybir.AluOpType
RG = [list(range(NCORES))]

_COMPILED = None


def _bf(x):
    return np.ascontiguousarray(np.asarray(x, dtype=np.float32)).astype(ml_dtypes.bfloat16)


def _f32(x):
    return np.ascontiguousarray(np.asarray(x, dtype=np.float32))


def _ktiles(W_slice, nk):
    """W_slice: [4*HS, K] gate-major rows. -> [128, nk*4*128]:
    sb[p, (j*4+g)*128 + m] = W_slice[g*HS + m, j*128 + p]."""
    K = W_slice.shape[1]
    assert K == nk * 128 and W_slice.shape[0] == 4 * HS
    G = W_slice.reshape(4, HS, K)   # [g, m, kin]
    X = G.transpose(2, 0, 1)        # [kin, g, m]
    X = X.reshape(nk, 128, 4, HS)   # [j, p, g, m]
    X = X.transpose(1, 0, 2, 3)     # [p, j, g, m]
    return np.ascontiguousarray(X.reshape(128, nk * 4 * HS))


def _rhs_tiles(WT):
    """WT: [K, N] -> [128, (K//128)*N], block j = WT[128j:128(j+1), :]."""
    K, N = WT.shape
    nk = K // 128
    X = WT.reshape(nk, 128, N).transpose(1, 0, 2)
    return np.ascontiguousarray(X.reshape(128, nk * N))


def prep_inputs(inputs):
    mem = _f32(inputs["memory"])
    dec = _f32(inputs["decoder_inputs"])
    mlen = np.asarray(inputs["memory_lengths"]).astype(np.int64)

    aW1 = _f32(inputs["attn_rnn_Wih"])
    aW2 = _f32(inputs["attn_rnn_Whh"])
    ab = _f32(inputs["attn_rnn_bih"]) + _f32(inputs["attn_rnn_bhh"])
    dW1 = _f32(inputs["dec_rnn_Wih"])
    dW2 = _f32(inputs["dec_rnn_Whh"])
    db = _f32(inputs["dec_rnn_bih"]) + _f32(inputs["dec_rnn_bhh"])
    qW = _f32(inputs["query_W"])
    mW = _f32(inputs["mem_W"])
    cW = _f32(inputs["loc_conv_W"])
    ldW = _f32(inputs["loc_dense_W"])
    av = _f32(inputs["attn_v"])
    pW1 = _f32(inputs["prenet_W1"])
    pb1 = _f32(inputs["prenet_b1"])
    pW2 = _f32(inputs["prenet_W2"])
    pb2 = _f32(inputs["prenet_b2"])
    prW = _f32(inputs["proj_W"])
    prb = _f32(inputs["proj_b"])
    gW = _f32(inputs["gate_W"])
    gb = _f32(inputs["gate_b"])

    frames = np.zeros((N_MEL, T_DEC, B), np.float32)
    frames[:, 1:, :] = dec.transpose(1, 2, 0)[:, : T_DEC - 1, :]
    framesT = _bf(frames.reshape(N_MEL, T_DEC * B))

    mfold = np.einsum("af,fcd->acd", ldW, cW)          # [128, 2, 31]
    mfold_sb = _bf(mfold.transpose(1, 2, 0).reshape(2 * LOCK, A))

    PG = np.concatenate([prW, gW], axis=0)             # [81, 1536]
    shared = {
        "framesT": framesT,
        "mfold": mfold_sb,
        "wpg": _bf(_rhs_tiles(PG.T)),
        "bias_pg": _f32(np.concatenate([prb, gb])[:, None]),
        "qwt": _bf(_rhs_tiles(qW.T)),
        "memw": _bf(_rhs_tiles(mW.T)),
        "vt": _bf(av.reshape(A, 1)),
        "w1p": _bf(pW1.T),
        "b1p": _f32(pb1.reshape(2, 128).T),
        "w2p": _bf(pW2.T.reshape(2, 128, 2, 128).transpose(1, 0, 2, 3).reshape(128, 512)),
        "b2p": _f32(pb2.reshape(2, 128).T),
        "ident": _bf(np.eye(128, dtype=np.float32)),
    }

    in_maps = []
    for k in range(NCORES):
        hs = slice(HS * k, HS * (k + 1))
        rows = np.concatenate(
            [np.arange(g * H + HS * k, g * H + HS * (k + 1)) for g in range(4)]
        )
        d = dict(shared)
        d["w1a"] = _bf(_ktiles(aW1[rows], 6))
        d["w2a"] = _bf(_ktiles(aW2[rows], 8))
        d["w1d"] = _bf(_ktiles(dW1[rows], 12))
        d["w2d"] = _bf(_ktiles(dW2[rows], 8))
        d["b_a"] = _f32(ab.reshape(4, H)[:, hs].T)
        d["b_d"] = _f32(db.reshape(4, H)[:, hs].T)
        ml = mem[2 * k : 2 * k + 2]                    # [2, 256, 512]
        d["mem_td"] = _bf(ml.reshape(2, 2, 128, E).transpose(2, 0, 1, 3).reshape(128, 2048))
        d["mem_dt"] = _bf(
            ml.transpose(2, 0, 1).reshape(4, 128, 2 * T_ENC).transpose(1, 0, 2).reshape(128, 2048)
        )
        d["mask01"] = _f32((np.arange(T_ENC)[None, :] < mlen[2 * k : 2 * k + 2, None]).reshape(1, 512))
        in_maps.append(d)
    return in_maps


_INPUT_SPECS = [
    ("w1a", (128, 6 * 4 * 128), BF16),
    ("w2a", (128, 8 * 4 * 128), BF16),
    ("w1d", (128, 12 * 4 * 128), BF16),
    ("w2d", (128, 8 * 4 * 128), BF16),
    ("b_a", (128, 4), F32),
    ("b_d", (128, 4), F32),
    ("qwt", (128, 8 * 128), BF16),
    ("memw", (128, 4 * 128), BF16),
    ("vt", (128, 1), BF16),
    ("mfold", (62, 128), BF16),
    ("wpg", (128, 12 * 81), BF16),
    ("bias_pg", (81, 1), F32),
    ("mask01", (1, 512), F32),
    ("framesT", (80, 1024), BF16),
    ("w1p", (80, 256), BF16),
    ("b1p", (128, 2), F32),
    ("w2p", (128, 512), BF16),
    ("b2p", (128, 2), F32),
    ("mem_td", (128, 2048), BF16),
    ("mem_dt", (128, 2048), BF16),
    ("ident", (128, 128), BF16),
]


def build(t_dec=T_DEC):
    nc = bacc.Bacc("TRN2", target_bir_lowering=False, debug=False, num_devices=NCORES)

    ins = {}
    for name, shape, dt in _INPUT_SPECS:
        ins[name] = nc.dram_tensor(name, list(shape), dt, kind="ExternalInput")

    out_mel = nc.dram_tensor("out_mel", [BLOC, N_MEL, t_dec], F32, kind="ExternalOutput")
    out_gate = nc.dram_tensor("out_gate", [BLOC, t_dec], F32, kind="ExternalOutput")
    out_align = nc.dram_tensor("out_align", [BLOC, t_dec, T_ENC], F32, kind="ExternalOutput")

    with tile.TileContext(nc) as tc:
        r2 = nc.partition_id() * BLOC

        with (
            tc.tile_pool(name="w", bufs=1) as wp,
            tc.tile_pool(name="st", bufs=1) as st,
            tc.tile_pool(name="lp", bufs=2) as lp,
            tc.tile_pool(name="ga", bufs=1, space="PSUM") as pga,
            tc.tile_pool(name="gd", bufs=1, space="PSUM") as pgd,
            tc.tile_pool(name="pmA", bufs=2, space="PSUM") as pmA,
            tc.tile_pool(name="pmB", bufs=2, space="PSUM") as pmB,
            tc.tile_pool(name="tr", bufs=2, space="PSUM") as ptr,
            tc.tile_pool(name="dram", bufs=2, space="DRAM") as dr,
        ):
            sb = {}
            for name, shape, dt in _INPUT_SPECS:
                t_ = wp.tile(list(shape), dt, name=f"sb_{name}")
                nc.sync.dma_start(t_[:], ins[name][:])
                sb[name] = t_

            # persistent state
            preT = st.tile([128, 2048], BF16, name="preT")
            pre1T = st.tile([128, 2048], BF16, name="pre1T")
            pmT = st.tile([128, 512], F32, name="pmT")
            ca = st.tile([128, 16], F32, name="ca")
            cd = st.tile([128, 16], F32, name="cd")
            awpad = st.tile([1, 2 * 286], BF16, name="awpad")
            cumpad = st.tile([1, 2 * 286], BF16, name="cumpad")
            awcum_f = st.tile([1, 512], F32, name="awcum_f")
            for t_ in (ca, cd, awpad, cumpad, awcum_f):
                nc.gpsimd.memset(t_[:], 0.0)

            # ---- prenet ----
            for m in range(2):
                for nch in range(2):
                    ps1 = pmA.tile([128, 512], F32, name="psA", tag="pmA")
                    nc.tensor.matmul(
                        ps1[:],
                        sb["w1p"][0:80, m * 128 : (m + 1) * 128],
                        sb["framesT"][0:80, nch * 512 : (nch + 1) * 512],
                        start=True, stop=True,
                    )
                    nc.scalar.activation(
                        pre1T[:, m * 1024 + nch * 512 : m * 1024 + (nch + 1) * 512],
                        ps1[:], AF.Relu, bias=sb["b1p"][:, m : m + 1],
                    )
            for m2 in range(2):
                for nch in range(2):
                    ps2 = pmA.tile([128, 512], F32, name="psA", tag="pmA")
                    for j in range(2):
                        nc.tensor.matmul(
                            ps2[:],
                            sb["w2p"][:, (j * 2 + m2) * 128 : (j * 2 + m2 + 1) * 128],
                            pre1T[:, j * 1024 + nch * 512 : j * 1024 + (nch + 1) * 512],
                            start=(j == 0), stop=(j == 1),
                        )
                    nc.scalar.activation(
                        preT[:, m2 * 1024 + nch * 512 : m2 * 1024 + (nch + 1) * 512],
                        ps2[:], AF.Relu, bias=sb["b2p"][:, m2 : m2 + 1],
                    )

            # ---- processed memory pmT[a, bl*256+t] ----
            ps3 = pmA.tile([128, 512], F32, name="psA", tag="pmA")
            for jd in range(4):
                nc.tensor.matmul(
                    ps3[:],
                    sb["memw"][:, jd * 128 : (jd + 1) * 128],
                    sb["mem_dt"][:, jd * 512 : (jd + 1) * 512],
                    start=(jd == 0), stop=(jd == 3),
                )
            nc.vector.tensor_copy(pmT[:], ps3[:])

            # zero-init loop-carried tiles
            hd_full = lp.tile([128, 256], BF16, name="hd_full")
            ctxT_full = lp.tile([128, 64], BF16, name="ctxT_full")
            dh_tr = lp.tile([16, 128], BF16, name="dh_tr")
            for t_ in (hd_full, ctxT_full, dh_tr):
                nc.gpsimd.memset(t_[:], 0.0)

            ctxT_prev = None   # ctxT_full of step t-1 (for proj at t-1)
            n_ag = 0

            def ctx_kt(ct, jd):
                v = ct[:, jd : jd + 1].copy()
                v.ap = mybir.VecI64Pair([[int(v.ap[0][0]), int(v.ap[0][1])], [4, 16]])
                return v

            def lstm_elem(gps, bias, c_state, out_bf):
                """gps: PSUM [128,64] gate tiles (i,f,g,o); updates c_state, writes h -> out_bf."""
                i_s = lp.tile([128, 16], F32, name="els_i", tag="els_i")
                f_s = lp.tile([128, 16], F32, name="els_f", tag="els_f")
                g_t = lp.tile([128, 16], F32, name="els_g", tag="els_g")
                o_s = lp.tile([128, 16], F32, name="els_o", tag="els_o")
                nc.scalar.activation(i_s[:], gps[:, 0:16], AF.Sigmoid, bias=bias[:, 0:1])
                nc.scalar.activation(f_s[:], gps[:, 16:32], AF.Sigmoid, bias=bias[:, 1:2])
                nc.scalar.activation(g_t[:], gps[:, 32:48], AF.Tanh, bias=bias[:, 2:3])
                nc.scalar.activation(o_s[:], gps[:, 48:64], AF.Sigmoid, bias=bias[:, 3:4])
                t1 = lp.tile([128, 16], F32, name="els_t1", tag="els_t1")
                t2 = lp.tile([128, 16], F32, name="els_t2", tag="els_t2")
                nc.vector.tensor_mul(t1[:], i_s[:], g_t[:])
                nc.vector.tensor_mul(t2[:], f_s[:], c_state[:])
                nc.vector.tensor_add(c_state[:], t1[:], t2[:])
                t3 = lp.tile([128, 16], F32, name="els_t3", tag="els_t3")
                nc.scalar.activation(t3[:], c_state[:], AF.Tanh)
                nc.vector.tensor_mul(out_bf[:], o_s[:], t3[:])

            for t in range(t_dec):
                # ============ attn LSTM gates ============
                ga = pga.tile([128, 64], F32, name="ga", tag="ga")
                for g in range(4):
                    for j in range(2):  # x_t part (K tiles 0,1 of cell_in)
                        nc.tensor.matmul(
                            ga[:, g * 16 : g * 16 + 16],
                            sb["w1a"][:, (j * 4 + g) * 128 : (j * 4 + g + 1) * 128],
                            preT[:, j * 1024 + 16 * t : j * 1024 + 16 * t + 16],
                            start=(j == 0), stop=False,
                        )
                    for jc in range(4):  # ctx(t-1) part (K tiles 2..5)
                        nc.tensor.matmul(
                            ga[:, g * 16 : g * 16 + 16],
                            sb["w1a"][:, ((jc + 2) * 4 + g) * 128 : ((jc + 2) * 4 + g + 1) * 128],
                            ctx_kt(ctxT_full, jc),
                            start=False, stop=False,
                        )
                    for j2 in range(8):  # ah(t-1) part
                        nc.tensor.matmul(
                            ga[:, g * 16 : g * 16 + 16],
                            sb["w2a"][:, (j2 * 4 + g) * 128 : (j2 * 4 + g + 1) * 128],
                            hd_full[:, j2 * 32 : j2 * 32 + 16],
                            start=False, stop=(j2 == 7),
                        )
                haT = lp.tile([128, 16], BF16, name="haT", tag="haT")
                lstm_elem(ga, sb["b_a"], ca, haT)

                # ============ AGa: gather (ah(t), dh(t-1)) ============
                tr_ah = ptr.tile([16, 128], BF16, name="tr_ah", tag="tr")
                nc.tensor.transpose(tr_ah[:], haT[:], sb["ident"][:])
                ah_tr = lp.tile([16, 128], BF16, name="ah_tr", tag="ah_tr")
                nc.vector.tensor_copy(ah_tr[:], tr_ah[:])

                aga_in = dr.tile([2, 16, 128], BF16, name="aga_in", tag="aga_in")
                aga_out = dr.tile([NCORES, 2, 16, 128], BF16, name="aga_out",
                                  tag="aga_out", addr_space="Shared")
                nc.sync.dma_start(aga_in[0], ah_tr[:])
                nc.sync.dma_start(aga_in[1], dh_tr[:])
                nc.gpsimd.collective_compute(
                    "AllGather", ALU.bypass, replica_groups=RG,
                    ins=[aga_in[:].opt()], outs=[aga_out[:].opt()],
                )
                n_ag += 1
                hd_full = lp.tile([128, 256], BF16, name="hd_full")
                src_a = aga_out[:].copy()
                src_a.ap = mybir.VecI64Pair([[1, 128], [128, 256]])
                nc.sync.dma_start(hd_full[:], src_a)

                # ============ location features (uses aw(t-1): off critical path) ====
                im = lp.tile([62, 512], BF16, name="im", tag="im")
                for c, padt in ((0, awpad), (1, cumpad)):
                    for bl in range(BLOC):
                        src = padt[0:1, bl * 286 : bl * 286 + 286].copy()
                        src.ap = mybir.VecI64Pair([[572, 1], [1, 31], [1, 256]])
                        nc.sync.dma_start(
                            im[c * 31 : (c + 1) * 31, bl * 256 : (bl + 1) * 256], src
                        )
                loc_ps = pmA.tile([128, 512], F32, name="psA", tag="pmA")
                nc.tensor.matmul(loc_ps[:], sb["mfold"][0:62, :], im[0:62, :],
                                 start=True, stop=True)

                # ============ query (after AGa) ============
                qT_ps = pmB.tile([128, 2], F32, name="psB", tag="pmB")
                for j in range(8):
                    nc.tensor.matmul(
                        qT_ps[:],
                        sb["qwt"][:, j * 128 : (j + 1) * 128],
                        hd_full[:, bass.ds(j * 32 + r2, 2)],
                        start=(j == 0), stop=(j == 7),
                    )
                qT = lp.tile([128, 2], F32, name="qT", tag="qT")
                nc.vector.tensor_copy(qT[:], qT_ps[:])

                # ============ energies + softmax ============
                xsum = lp.tile([128, 512], F32, name="xsum", tag="xsum")
                for bl in range(BLOC):
                    nc.vector.scalar_tensor_tensor(
                        xsum[:, bl * 256 : (bl + 1) * 256],
                        loc_ps[:, bl * 256 : (bl + 1) * 256],
                        qT[:, bl : bl + 1],
                        pmT[:, bl * 256 : (bl + 1) * 256],
                        op0=ALU.add, op1=ALU.add,
                    )
                xt = lp.tile([128, 512], BF16, name="xt", tag="xt")
                nc.scalar.activation(xt[:], xsum[:], AF.Tanh)
                e_ps = pmB.tile([1, 512], F32, name="psB", tag="pmB")
                nc.tensor.matmul(e_ps[:], sb["vt"][:, 0:1], xt[:], start=True, stop=True)
                aw_un = lp.tile([1, 512], F32, name="aw_un", tag="aw_un")
                nc.scalar.activation(aw_un[:], e_ps[:], AF.Exp)
                nc.vector.tensor_mul(aw_un[:], aw_un[:], sb["mask01"][:])
                zsum = lp.tile([1, 2], F32, name="zsum", tag="zsum")
                zin = aw_un[0:1, :].copy()
                zin.ap = mybir.VecI64Pair([[512, 1], [256, 2], [1, 256]])
                nc.vector.tensor_reduce(zsum[0:1, :].unsqueeze(-1), zin,
                                        axis=mybir.AxisListType.X, op=ALU.add)
                rz = lp.tile([1, 2], F32, name="rz", tag="rz")
                nc.vector.reciprocal(rz[:], zsum[:])
                aw_f = lp.tile([1, 512], F32, name="aw_f", tag="aw_f")
                for bl in range(BLOC):
                    nc.vector.tensor_scalar_mul(
                        aw_f[0:1, bl * 256 : (bl + 1) * 256],
                        aw_un[0:1, bl * 256 : (bl + 1) * 256],
                        rz[0:1, bl : bl + 1],
                    )
                # outputs + state updates
                valign = out_align[0, t, :].copy()
                valign.ap = mybir.VecI64Pair([[T_DEC * T_ENC, 2], [1, 256]])
                nc.sync.dma_start(valign, aw_f[:])
                nc.vector.tensor_add(awcum_f[:], awcum_f[:], aw_f[:])
                aw_bf = lp.tile([1, 512], BF16, name="aw_bf", tag="aw_bf")
                nc.vector.tensor_copy(aw_bf[:], aw_f[:])
                for bl in range(BLOC):
                    nc.vector.tensor_copy(
                        awpad[0:1, bl * 286 + 15 : bl * 286 + 271],
                        aw_bf[0:1, bl * 256 : (bl + 1) * 256])
                    nc.vector.tensor_copy(
                        cumpad[0:1, bl * 286 + 15 : bl * 286 + 271],
                        awcum_f[0:1, bl * 256 : (bl + 1) * 256])

                # ============ ctx ============
                awT_ps = ptr.tile([128, 8], BF16, name="awT_ps", tag="tr")
                for bl in range(BLOC):
                    for jt in range(2):
                        c = (jt * 2 + bl) * 2
                        nc.tensor.transpose(
                            awT_ps[:, c : c + 1],
                            aw_bf[0:1, bl * 256 + jt * 128 : bl * 256 + (jt + 1) * 128],
                            sb["ident"][0:1, 0:1],
                        )
                awT = lp.tile([128, 8], BF16, name="awT", tag="awT")
                nc.vector.tensor_copy(awT[:], awT_ps[:])
                ctx_sb = lp.tile([1, 1024], BF16, name="ctx_sb", tag="ctx_sb")
                for bl in range(BLOC):
                    ctx_ps = pmB.tile([1, 512], F32, name="psB", tag="pmB")
                    for jt in range(2):
                        nc.tensor.matmul(
                            ctx_ps[:],
                            awT[:, (jt * 2 + bl) * 2 : (jt * 2 + bl) * 2 + 1],
                            sb["mem_td"][:, (bl * 2 + jt) * 512 : (bl * 2 + jt + 1) * 512],
                            start=(jt == 0), stop=(jt == 1),
                        )
                    nc.vector.tensor_copy(ctx_sb[0:1, bl * 512 : (bl + 1) * 512], ctx_ps[:])

                # ============ AGc: gather ctx ============
                agc_in = dr.tile([1, 1024], BF16, name="agc_in", tag="agc_in")
                agc_out = dr.tile([B, 512], BF16, name="agc_out", tag="agc_out",
                                  addr_space="Shared")
                nc.sync.dma_start(agc_in[:], ctx_sb[:])
                nc.gpsimd.collective_compute(
                    "AllGather", ALU.bypass, replica_groups=RG,
                    ins=[agc_in[:].opt()], outs=[agc_out[:].opt()],
                )
                n_ag += 1
                ctxT_prev = ctxT_full
                ctxT_full = lp.tile([128, 64], BF16, name="ctxT_full")
                src_c = agc_out[:].copy()
                src_c.ap = mybir.VecI64Pair([[1, 128], [128, 64]])
                nc.sync.dma_start(ctxT_full[:], src_c)

                # ============ proj head for step t-1 (needs dh(t-1) = dhT_full) ====
                if t > 0:
                    pg_ps = pmB.tile([81, 2], F32, name="psB", tag="pmB")
                    for bl in range(BLOC):
                        for j in range(8):
                            nc.tensor.matmul(
                                pg_ps[:, bl : bl + 1],
                                sb["wpg"][:, j * 81 : (j + 1) * 81],
                                hd_full[:, bass.ds(j * 32 + 16 + r2 + bl, 1)],
                                start=(j == 0), stop=False,
                            )
                        for jc in range(4):
                            nc.tensor.matmul(
                                pg_ps[:, bl : bl + 1],
                                sb["wpg"][:, (8 + jc) * 81 : (9 + jc) * 81],
                                ctxT_prev[:, bass.ds((r2 + bl) * 4 + jc, 1)],
                                start=False, stop=(jc == 3),
                            )
                    pg_sb = lp.tile([81, 2], F32, name="pg_sb", tag="pg_sb")
                    nc.vector.tensor_scalar_add(pg_sb[:], pg_ps[:], sb["bias_pg"][:, 0:1])
                    vmel = out_mel[0, 0, t - 1 : t].copy()
                    vmel.ap = mybir.VecI64Pair([[64, 80], [80 * 64, 2]])
                    nc.sync.dma_start(vmel, pg_sb[0:80, 0:2])
                    vgate = out_gate[0, t - 1 : t].copy()
                    vgate.ap = mybir.VecI64Pair([[64, 2]])
                    nc.sync.dma_start(vgate, pg_sb[80:81, 0:2])

                # ============ dec LSTM ============
                gd = pgd.tile([128, 64], F32, name="gd", tag="gd")
                for g in range(4):
                    for j in range(8):  # ah part
                        nc.tensor.matmul(
                            gd[:, g * 16 : g * 16 + 16],
                            sb["w1d"][:, (j * 4 + g) * 128 : (j * 4 + g + 1) * 128],
                            hd_full[:, j * 32 : j * 32 + 16],
                            start=(j == 0), stop=False,
                        )
                    for j2 in range(8):  # dh(t-1) part
                        nc.tensor.matmul(
                            gd[:, g * 16 : g * 16 + 16],
                            sb["w2d"][:, (j2 * 4 + g) * 128 : (j2 * 4 + g + 1) * 128],
                            hd_full[:, j2 * 32 + 16 : j2 * 32 + 32],
                            start=False, stop=False,
                        )
                    for jc in range(4):  # ctx(t) part
                        nc.tensor.matmul(
                            gd[:, g * 16 : g * 16 + 16],
                            sb["w1d"][:, ((8 + jc) * 4 + g) * 128 : ((8 + jc) * 4 + g + 1) * 128],
                            ctx_kt(ctxT_full, jc),
                            start=False, stop=(jc == 3),
                        )
                hdT = lp.tile([128, 16], BF16, name="hdT", tag="hdT")
                lstm_elem(gd, sb["b_d"], cd, hdT)
                tr_dh = ptr.tile([16, 128], BF16, name="tr_dh", tag="tr")
                nc.tensor.transpose(tr_dh[:], hdT[:], sb["ident"][:])
                dh_tr = lp.tile([16, 128], BF16, name="dh_tr")
                nc.vector.tensor_copy(dh_tr[:], tr_dh[:])

            # ============ epilogue: gather dh(T-1), final proj ============
            agf_in = dr.tile([16, 128], BF16, name="agf_in", tag="agf_in")
            agf_out = dr.tile([NCORES, 16, 128], BF16, name="agf_out", tag="agf_out",
                              addr_space="Shared")
            nc.sync.dma_start(agf_in[:], dh_tr[:])
            nc.gpsimd.collective_compute(
                "AllGather", ALU.bypass, replica_groups=RG,
                ins=[agf_in[:].opt()], outs=[agf_out[:].opt()],
            )
            dhT_fin = lp.tile([128, 128], BF16, name="dhT_full")
            src_f = agf_out[:].copy()
            src_f.ap = mybir.VecI64Pair([[1, 128], [128, 128]])
            nc.sync.dma_start(dhT_fin[:], src_f)
            pg_ps = pmB.tile([81, 2], F32, name="psB", tag="pmB")
            for bl in range(BLOC):
                for j in range(8):
                    nc.tensor.matmul(
                        pg_ps[:, bl : bl + 1], sb["wpg"][:, j * 81 : (j + 1) * 81],
                        dhT_fin[:, bass.ds(j * 16 + r2 + bl, 1)],
                        start=(j == 0), stop=False,
                    )
                for jc in range(4):
                    nc.tensor.matmul(
                        pg_ps[:, bl : bl + 1], sb["wpg"][:, (8 + jc) * 81 : (9 + jc) * 81],
                        ctxT_full[:, bass.ds((r2 + bl) * 4 + jc, 1)],
                        start=False, stop=(jc == 3),
                    )
            pg_sb = lp.tile([81, 2], F32, name="pg_sb", tag="pg_sb")
            nc.vector.tensor_scalar_add(pg_sb[:], pg_ps[:], sb["bias_pg"][:, 0:1])
            vmel = out_mel[0, 0, t_dec - 1 : t_dec].copy()
            vmel.ap = mybir.VecI64Pair([[64, 80], [80 * 64, 2]])
            nc.sync.dma_start(vmel, pg_sb[0:80, 0:2])
            vgate = out_gate[0, t_dec - 1 : t_dec].copy()
            vgate.ap = mybir.VecI64Pair([[64, 2]])
            nc.sync.dma_start(vgate, pg_sb[80:81, 0:2])

    nc.compile()
    return nc


def kernel(**inputs):
    global _COMPILED
    if _COMPILED is None:
        _COMPILED = build()
    nc = _COMPILED
    in_maps = prep_inputs(inputs)
    res = bass_utils.run_bass_kernel_spmd(nc, in_maps, core_ids=list(range(NCORES)))
    mel = np.concatenate([res.results[k]["out_mel"] for k in range(NCORES)], axis=0)
    gate = np.concatenate([res.results[k]["out_gate"] for k in range(NCORES)], axis=0)
    align = np.concatenate([res.results[k]["out_align"] for k in range(NCORES)], axis=0)
    return mel.astype(np.float32), gate.astype(np.float32), align.astype(np.float32)


# revision 9
# speedup vs baseline: 1.8584x; 1.8584x over previous
"""Tacotron2-style decoder on 8 Trainium2 NeuronCores.

Sharding strategy:
- Both LSTM cells sharded 8-way over the hidden dim (each core owns 128 of 1024
  hidden units = 512 of 4096 gate rows per weight matrix). Weights stay
  SBUF-resident in bf16 (~4.5 MB/core) instead of streaming ~71 MB/step.
- Location-sensitive attention sharded over batch (2 of 16 per core).
- Two AllGathers per step via DRAM bounce buffers: (ah(t), dh(t-1)) combined,
  and ctx(t). Activations are kept feature-on-partition ("transposed") so LSTM
  elementwise runs 128 lanes wide and gather loads are contiguous-run DMAs.

kernel(**inputs) takes full unsharded inputs (as reference.setup_inputs) and
returns (mel_outputs [16,80,64], gate_outputs [16,64], alignments [16,64,256]).
"""
import numpy as np
import ml_dtypes

import concourse.bass as bass
import concourse.bacc as bacc
import concourse.tile as tile
from concourse import mybir
from concourse import bass_utils

NCORES = 8
B, T_ENC, T_DEC = 16, 256, 64
N_MEL, E, H = 80, 512, 1024
A = 128
LOCK = 31
BLOC = B // NCORES  # 2 local batches per core
HS = H // NCORES    # 128 hidden units per core

BF16 = mybir.dt.bfloat16
F32 = mybir.dt.float32
AF = mybir.ActivationFunctionType
ALU = mybir.AluOpType
RG = [list(range(NCORES))]

_COMPILED = None


def _bf(x):
    return np.ascontiguousarray(np.asarray(x, dtype=np.float32)).astype(ml_dtypes.bfloat16)


def _f32(x):
    return np.ascontiguousarray(np.asarray(x, dtype=np.float32))


def _ktiles(W_slice, nk):
    """W_slice: [4*HS, K] gate-major rows. -> [128, nk*4*128]:
    sb[p, (j*4+g)*128 + m] = W_slice[g*HS + m, j*128 + p]."""
    K = W_slice.shape[1]
    assert K == nk * 128 and W_slice.shape[0] == 4 * HS
    G = W_slice.reshape(4, HS, K)   # [g, m, kin]
    X = G.transpose(2, 0, 1)        # [kin, g, m]
    X = X.reshape(nk, 128, 4, HS)   # [j, p, g, m]
    X = X.transpose(1, 0, 2, 3)     # [p, j, g, m]
    return np.ascontiguousarray(X.reshape(128, nk * 4 * HS))


def _rhs_tiles(WT):
    """WT: [K, N] -> [128, (K//128)*N], block j = WT[128j:128(j+1), :]."""
    K, N = WT.shape
    nk = K // 128
    X = WT.reshape(nk, 128, N).transpose(1, 0, 2)
    return np.ascontiguousarray(X.reshape(128, nk * N))


def prep_inputs(inputs):
    mem = _f32(inputs["memory"])
    dec = _f32(inputs["decoder_inputs"])
    mlen = np.asarray(inputs["memory_lengths"]).astype(np.int64)

    aW1 = _f32(inputs["attn_rnn_Wih"])
    aW2 = _f32(inputs["attn_rnn_Whh"])
    ab = _f32(inputs["attn_rnn_bih"]) + _f32(inputs["attn_rnn_bhh"])
    dW1 = _f32(inputs["dec_rnn_Wih"])
    dW2 = _f32(inputs["dec_rnn_Whh"])
    db = _f32(inputs["dec_rnn_bih"]) + _f32(inputs["dec_rnn_bhh"])
    qW = _f32(inputs["query_W"])
    mW = _f32(inputs["mem_W"])
    cW = _f32(inputs["loc_conv_W"])
    ldW = _f32(inputs["loc_dense_W"])
    av = _f32(inputs["attn_v"])
    pW1 = _f32(inputs["prenet_W1"])
    pb1 = _f32(inputs["prenet_b1"])
    pW2 = _f32(inputs["prenet_W2"])
    pb2 = _f32(inputs["prenet_b2"])
    prW = _f32(inputs["proj_W"])
    prb = _f32(inputs["proj_b"])
    gW = _f32(inputs["gate_W"])
    gb = _f32(inputs["gate_b"])

    frames = np.zeros((N_MEL, T_DEC, B), np.float32)
    frames[:, 1:, :] = dec.transpose(1, 2, 0)[:, : T_DEC - 1, :]
    framesT = _bf(frames.reshape(N_MEL, T_DEC * B))

    mfold = np.einsum("af,fcd->acd", ldW, cW)          # [128, 2, 31]
    mfold_sb = _bf(mfold.transpose(1, 2, 0).reshape(2 * LOCK, A))

    PG = np.concatenate([prW, gW], axis=0)             # [81, 1536]
    shared = {
        "framesT": framesT,
        "mfold": mfold_sb,
        "wpg": _bf(_rhs_tiles(PG.T)),
        "bias_pg": _f32(np.concatenate([prb, gb])[:, None]),
        "qwt": _bf(_rhs_tiles(qW.T)),
        "memw": _bf(_rhs_tiles(mW.T)),
        "vt": _bf(av.reshape(A, 1)),
        "w1p": _bf(pW1.T),
        "b1p": _f32(pb1.reshape(2, 128).T),
        "w2p": _bf(pW2.T.reshape(2, 128, 2, 128).transpose(1, 0, 2, 3).reshape(128, 512)),
        "b2p": _f32(pb2.reshape(2, 128).T),
        "ident": _bf(np.eye(128, dtype=np.float32)),
    }

    in_maps = []
    for k in range(NCORES):
        hs = slice(HS * k, HS * (k + 1))
        rows = np.concatenate(
            [np.arange(g * H + HS * k, g * H + HS * (k + 1)) for g in range(4)]
        )
        d = dict(shared)
        d["w1a"] = _bf(_ktiles(aW1[rows], 6))
        d["w2a"] = _bf(_ktiles(aW2[rows], 8))
        d["w1d"] = _bf(_ktiles(dW1[rows], 12))
        d["w2d"] = _bf(_ktiles(dW2[rows], 8))
        d["b_a"] = _f32(ab.reshape(4, H)[:, hs].T)
        d["b_d"] = _f32(db.reshape(4, H)[:, hs].T)
        ml = mem[2 * k : 2 * k + 2]                    # [2, 256, 512]
        d["mem_td"] = _bf(ml.reshape(2, 2, 128, E).transpose(2, 0, 1, 3).reshape(128, 2048))
        d["mem_dt"] = _bf(
            ml.transpose(2, 0, 1).reshape(4, 128, 2 * T_ENC).transpose(1, 0, 2).reshape(128, 2048)
        )
        d["mask01"] = _f32((np.arange(T_ENC)[None, :] < mlen[2 * k : 2 * k + 2, None]).reshape(1, 512))
        in_maps.append(d)
    return in_maps


_INPUT_SPECS = [
    ("w1a", (128, 6 * 4 * 128), BF16),
    ("w2a", (128, 8 * 4 * 128), BF16),
    ("w1d", (128, 12 * 4 * 128), BF16),
    ("w2d", (128, 8 * 4 * 128), BF16),
    ("b_a", (128, 4), F32),
    ("b_d", (128, 4), F32),
    ("qwt", (128, 8 * 128), BF16),
    ("memw", (128, 4 * 128), BF16),
    ("vt", (128, 1), BF16),
    ("mfold", (62, 128), BF16),
    ("wpg", (128, 12 * 81), BF16),
    ("bias_pg", (81, 1), F32),
    ("mask01", (1, 512), F32),
    ("framesT", (80, 1024), BF16),
    ("w1p", (80, 256), BF16),
    ("b1p", (128, 2), F32),
    ("w2p", (128, 512), BF16),
    ("b2p", (128, 2), F32),
    ("mem_td", (128, 2048), BF16),
    ("mem_dt", (128, 2048), BF16),
    ("ident", (128, 128), BF16),
]


def build(t_dec=T_DEC):
    nc = bacc.Bacc("TRN2", target_bir_lowering=False, debug=False, num_devices=NCORES)

    ins = {}
    for name, shape, dt in _INPUT_SPECS:
        ins[name] = nc.dram_tensor(name, list(shape), dt, kind="ExternalInput")

    out_mel = nc.dram_tensor("out_mel", [BLOC, N_MEL, t_dec], F32, kind="ExternalOutput")
    out_gate = nc.dram_tensor("out_gate", [BLOC, t_dec], F32, kind="ExternalOutput")
    out_align = nc.dram_tensor("out_align", [BLOC, t_dec, T_ENC], F32, kind="ExternalOutput")

    with tile.TileContext(nc) as tc:
        r2 = nc.partition_id() * BLOC

        with (
            tc.tile_pool(name="w", bufs=1) as wp,
            tc.tile_pool(name="st", bufs=1) as st,
            tc.tile_pool(name="lp", bufs=2) as lp,
            tc.tile_pool(name="ga", bufs=1, space="PSUM") as pga,
            tc.tile_pool(name="gd", bufs=1, space="PSUM") as pgd,
            tc.tile_pool(name="pmA", bufs=2, space="PSUM") as pmA,
            tc.tile_pool(name="pmB", bufs=2, space="PSUM") as pmB,
            tc.tile_pool(name="tr", bufs=2, space="PSUM") as ptr,
            tc.tile_pool(name="dram", bufs=2, space="DRAM") as dr,
        ):
            sb = {}
            for name, shape, dt in _INPUT_SPECS:
                t_ = wp.tile(list(shape), dt, name=f"sb_{name}")
                nc.sync.dma_start(t_[:], ins[name][:])
                sb[name] = t_

            # persistent state
            preT = st.tile([128, 2048], BF16, name="preT")
            pre1T = st.tile([128, 2048], BF16, name="pre1T")
            pmT = st.tile([128, 512], F32, name="pmT")
            ca = st.tile([128, 16], F32, name="ca")
            cd = st.tile([128, 16], F32, name="cd")
            awpad = st.tile([1, 2 * 286], BF16, name="awpad")
            cumpad = st.tile([1, 2 * 286], BF16, name="cumpad")
            awcum_f = st.tile([1, 512], F32, name="awcum_f")
            for t_ in (ca, cd, awpad, cumpad, awcum_f):
                nc.gpsimd.memset(t_[:], 0.0)

            # ---- prenet ----
            for m in range(2):
                for nch in range(2):
                    ps1 = pmA.tile([128, 512], F32, name="psA", tag="pmA")
                    nc.tensor.matmul(
                        ps1[:],
                        sb["w1p"][0:80, m * 128 : (m + 1) * 128],
                        sb["framesT"][0:80, nch * 512 : (nch + 1) * 512],
                        start=True, stop=True,
                    )
                    nc.scalar.activation(
                        pre1T[:, m * 1024 + nch * 512 : m * 1024 + (nch + 1) * 512],
                        ps1[:], AF.Relu, bias=sb["b1p"][:, m : m + 1],
                    )
            for m2 in range(2):
                for nch in range(2):
                    ps2 = pmA.tile([128, 512], F32, name="psA", tag="pmA")
                    for j in range(2):
                        nc.tensor.matmul(
                            ps2[:],
                            sb["w2p"][:, (j * 2 + m2) * 128 : (j * 2 + m2 + 1) * 128],
                            pre1T[:, j * 1024 + nch * 512 : j * 1024 + (nch + 1) * 512],
                            start=(j == 0), stop=(j == 1),
                        )
                    nc.scalar.activation(
                        preT[:, m2 * 1024 + nch * 512 : m2 * 1024 + (nch + 1) * 512],
                        ps2[:], AF.Relu, bias=sb["b2p"][:, m2 : m2 + 1],
                    )

            # ---- processed memory pmT[a, bl*256+t] ----
            ps3 = pmA.tile([128, 512], F32, name="psA", tag="pmA")
            for jd in range(4):
                nc.tensor.matmul(
                    ps3[:],
                    sb["memw"][:, jd * 128 : (jd + 1) * 128],
                    sb["mem_dt"][:, jd * 512 : (jd + 1) * 512],
                    start=(jd == 0), stop=(jd == 3),
                )
            nc.vector.tensor_copy(pmT[:], ps3[:])

            # zero-init loop-carried tiles
            hd_full = lp.tile([128, 256], BF16, name="hd_full")
            ctxT_full = lp.tile([128, 64], BF16, name="ctxT_full")
            dh_tr = lp.tile([16, 128], BF16, name="dh_tr")
            for t_ in (hd_full, ctxT_full, dh_tr):
                nc.gpsimd.memset(t_[:], 0.0)

            ctxT_prev = None   # ctxT_full of step t-1 (for proj at t-1)
            n_ag = 0

            def ctx_kt(ct, jd):
                v = ct[:, jd : jd + 1].copy()
                v.ap = mybir.VecI64Pair([[int(v.ap[0][0]), int(v.ap[0][1])], [4, 16]])
                return v

            def lstm_elem(gps, bias, c_state, out_bf):
                """gps: PSUM [128,64] gate tiles (i,f,g,o); updates c_state, writes h -> out_bf."""
                i_s = lp.tile([128, 16], F32, name="els_i", tag="els_i")
                f_s = lp.tile([128, 16], F32, name="els_f", tag="els_f")
                g_t = lp.tile([128, 16], F32, name="els_g", tag="els_g")
                o_s = lp.tile([128, 16], F32, name="els_o", tag="els_o")
                nc.scalar.activation(i_s[:], gps[:, 0:16], AF.Sigmoid, bias=bias[:, 0:1])
                nc.scalar.activation(f_s[:], gps[:, 16:32], AF.Sigmoid, bias=bias[:, 1:2])
                nc.scalar.activation(g_t[:], gps[:, 32:48], AF.Tanh, bias=bias[:, 2:3])
                nc.scalar.activation(o_s[:], gps[:, 48:64], AF.Sigmoid, bias=bias[:, 3:4])
                t1 = lp.tile([128, 16], F32, name="els_t1", tag="els_t1")
                t2 = lp.tile([128, 16], F32, name="els_t2", tag="els_t2")
                nc.vector.tensor_mul(t1[:], i_s[:], g_t[:])
                nc.vector.tensor_mul(t2[:], f_s[:], c_state[:])
                nc.vector.tensor_add(c_state[:], t1[:], t2[:])
                t3 = lp.tile([128, 16], F32, name="els_t3", tag="els_t3")
                nc.scalar.activation(t3[:], c_state[:], AF.Tanh)
                nc.vector.tensor_mul(out_bf[:], o_s[:], t3[:])

            for t in range(t_dec):
                # ============ attn LSTM gates ============
                ga = pga.tile([128, 64], F32, name="ga", tag="ga")
                for g in range(4):
                    for j in range(2):  # x_t part (K tiles 0,1 of cell_in)
                        nc.tensor.matmul(
                            ga[:, g * 16 : g * 16 + 16],
                            sb["w1a"][:, (j * 4 + g) * 128 : (j * 4 + g + 1) * 128],
                            preT[:, j * 1024 + 16 * t : j * 1024 + 16 * t + 16],
                            start=(j == 0), stop=False,
                        )
                    for jc in range(4):  # ctx(t-1) part (K tiles 2..5)
                        nc.tensor.matmul(
                            ga[:, g * 16 : g * 16 + 16],
                            sb["w1a"][:, ((jc + 2) * 4 + g) * 128 : ((jc + 2) * 4 + g + 1) * 128],
                            ctx_kt(ctxT_full, jc),
                            start=False, stop=False,
                        )
                    for j2 in range(8):  # ah(t-1) part
                        nc.tensor.matmul(
                            ga[:, g * 16 : g * 16 + 16],
                            sb["w2a"][:, (j2 * 4 + g) * 128 : (j2 * 4 + g + 1) * 128],
                            hd_full[:, j2 * 32 : j2 * 32 + 16],
                            start=False, stop=(j2 == 7),
                        )
                haT = lp.tile([128, 16], BF16, name="haT", tag="haT")
                lstm_elem(ga, sb["b_a"], ca, haT)

                # ============ AGa: gather (ah(t), dh(t-1)) ============
                tr_ah = ptr.tile([16, 128], BF16, name="tr_ah", tag="tr")
                nc.tensor.transpose(tr_ah[:], haT[:], sb["ident"][:])
                ah_tr = lp.tile([16, 128], BF16, name="ah_tr", tag="ah_tr")
                nc.vector.tensor_copy(ah_tr[:], tr_ah[:])

                aga_in = dr.tile([2, 16, 128], BF16, name="aga_in", tag="aga_in")
                aga_out = dr.tile([NCORES, 2, 16, 128], BF16, name="aga_out",
                                  tag="aga_out", addr_space="Shared")
                nc.sync.dma_start(aga_in[0], ah_tr[:])
                nc.sync.dma_start(aga_in[1], dh_tr[:])
                nc.gpsimd.collective_compute(
                    "AllGather", ALU.bypass, replica_groups=RG,
                    ins=[aga_in[:].opt()], outs=[aga_out[:].opt()],
                )
                n_ag += 1
                hd_full = lp.tile([128, 256], BF16, name="hd_full")
                nc.sync.dma_start_transpose(
                    hd_full[:], aga_out[:].rearrange("j h b p -> (j h b) p"))

                # ============ location features (uses aw(t-1): off critical path) ====
                im = lp.tile([62, 512], BF16, name="im", tag="im")
                for c, padt in ((0, awpad), (1, cumpad)):
                    for bl in range(BLOC):
                        src = padt[0:1, bl * 286 : bl * 286 + 286].copy()
                        src.ap = mybir.VecI64Pair([[572, 1], [1, 31], [1, 256]])
                        nc.sync.dma_start(
                            im[c * 31 : (c + 1) * 31, bl * 256 : (bl + 1) * 256], src
                        )
                loc_ps = pmA.tile([128, 512], F32, name="psA", tag="pmA")
                nc.tensor.matmul(loc_ps[:], sb["mfold"][0:62, :], im[0:62, :],
                                 start=True, stop=True)

                # ============ query (after AGa) ============
                qT_ps = pmB.tile([128, 2], F32, name="psB", tag="pmB")
                for j in range(8):
                    nc.tensor.matmul(
                        qT_ps[:],
                        sb["qwt"][:, j * 128 : (j + 1) * 128],
                        hd_full[:, bass.ds(j * 32 + r2, 2)],
                        start=(j == 0), stop=(j == 7),
                    )
                qT = lp.tile([128, 2], F32, name="qT", tag="qT")
                nc.vector.tensor_copy(qT[:], qT_ps[:])

                # ============ energies + softmax ============
                xsum = lp.tile([128, 512], F32, name="xsum", tag="xsum")
                for bl in range(BLOC):
                    nc.vector.scalar_tensor_tensor(
                        xsum[:, bl * 256 : (bl + 1) * 256],
                        loc_ps[:, bl * 256 : (bl + 1) * 256],
                        qT[:, bl : bl + 1],
                        pmT[:, bl * 256 : (bl + 1) * 256],
                        op0=ALU.add, op1=ALU.add,
                    )
                xt = lp.tile([128, 512], BF16, name="xt", tag="xt")
                nc.scalar.activation(xt[:], xsum[:], AF.Tanh)
                e_ps = pmB.tile([1, 512], F32, name="psB", tag="pmB")
                nc.tensor.matmul(e_ps[:], sb["vt"][:, 0:1], xt[:], start=True, stop=True)
                aw_un = lp.tile([1, 512], F32, name="aw_un", tag="aw_un")
                nc.scalar.activation(aw_un[:], e_ps[:], AF.Exp)
                nc.vector.tensor_mul(aw_un[:], aw_un[:], sb["mask01"][:])
                zsum = lp.tile([1, 2], F32, name="zsum", tag="zsum")
                zin = aw_un[0:1, :].copy()
                zin.ap = mybir.VecI64Pair([[512, 1], [256, 2], [1, 256]])
                nc.vector.tensor_reduce(zsum[0:1, :].unsqueeze(-1), zin,
                                        axis=mybir.AxisListType.X, op=ALU.add)
                rz = lp.tile([1, 2], F32, name="rz", tag="rz")
                nc.vector.reciprocal(rz[:], zsum[:])
                aw_f = lp.tile([1, 512], F32, name="aw_f", tag="aw_f")
                for bl in range(BLOC):
                    nc.vector.tensor_scalar_mul(
                        aw_f[0:1, bl * 256 : (bl + 1) * 256],
                        aw_un[0:1, bl * 256 : (bl + 1) * 256],
                        rz[0:1, bl : bl + 1],
                    )
                # outputs + state updates
                valign = out_align[0, t, :].copy()
                valign.ap = mybir.VecI64Pair([[T_DEC * T_ENC, 2], [1, 256]])
                nc.sync.dma_start(valign, aw_f[:])
                nc.vector.tensor_add(awcum_f[:], awcum_f[:], aw_f[:])
                aw_bf = lp.tile([1, 512], BF16, name="aw_bf", tag="aw_bf")
                nc.vector.tensor_copy(aw_bf[:], aw_f[:])
                for bl in range(BLOC):
                    nc.vector.tensor_copy(
                        awpad[0:1, bl * 286 + 15 : bl * 286 + 271],
                        aw_bf[0:1, bl * 256 : (bl + 1) * 256])
                    nc.vector.tensor_copy(
                        cumpad[0:1, bl * 286 + 15 : bl * 286 + 271],
                        awcum_f[0:1, bl * 256 : (bl + 1) * 256])

                # ============ ctx ============
                awT_ps = ptr.tile([128, 8], BF16, name="awT_ps", tag="tr")
                for bl in range(BLOC):
                    for jt in range(2):
                        c = (jt * 2 + bl) * 2
                        nc.tensor.transpose(
                            awT_ps[:, c : c + 1],
                            aw_bf[0:1, bl * 256 + jt * 128 : bl * 256 + (jt + 1) * 128],
                            sb["ident"][0:1, 0:1],
                        )
                awT = lp.tile([128, 8], BF16, name="awT", tag="awT")
                nc.vector.tensor_copy(awT[:], awT_ps[:])
                ctx_sb = lp.tile([1, 1024], BF16, name="ctx_sb", tag="ctx_sb")
                for bl in range(BLOC):
                    ctx_ps = pmB.tile([1, 512], F32, name="psB", tag="pmB")
                    for jt in range(2):
                        nc.tensor.matmul(
                            ctx_ps[:],
                            awT[:, (jt * 2 + bl) * 2 : (jt * 2 + bl) * 2 + 1],
                            sb["mem_td"][:, (bl * 2 + jt) * 512 : (bl * 2 + jt + 1) * 512],
                            start=(jt == 0), stop=(jt == 1),
                        )
                    nc.vector.tensor_copy(ctx_sb[0:1, bl * 512 : (bl + 1) * 512], ctx_ps[:])

                # ============ AGc: gather ctx ============
                agc_in = dr.tile([1, 1024], BF16, name="agc_in", tag="agc_in")
                agc_out = dr.tile([B, 512], BF16, name="agc_out", tag="agc_out",
                                  addr_space="Shared")
                nc.sync.dma_start(agc_in[:], ctx_sb[:])
                nc.gpsimd.collective_compute(
                    "AllGather", ALU.bypass, replica_groups=RG,
                    ins=[agc_in[:].opt()], outs=[agc_out[:].opt()],
                )
                n_ag += 1
                ctxT_prev = ctxT_full
                ctxT_full = lp.tile([128, 64], BF16, name="ctxT_full")
                nc.sync.dma_start_transpose(
                    ctxT_full[:], agc_out[:].rearrange("b (jd p) -> (b jd) p", p=128))

                # ============ proj head for step t-1 (needs dh(t-1) = dhT_full) ====
                if t > 0:
                    pg_ps = pmB.tile([81, 2], F32, name="psB", tag="pmB")
                    for bl in range(BLOC):
                        for j in range(8):
                            nc.tensor.matmul(
                                pg_ps[:, bl : bl + 1],
                                sb["wpg"][:, j * 81 : (j + 1) * 81],
                                hd_full[:, bass.ds(j * 32 + 16 + r2 + bl, 1)],
                                start=(j == 0), stop=False,
                            )
                        for jc in range(4):
                            nc.tensor.matmul(
                                pg_ps[:, bl : bl + 1],
                                sb["wpg"][:, (8 + jc) * 81 : (9 + jc) * 81],
                                ctxT_prev[:, bass.ds((r2 + bl) * 4 + jc, 1)],
                                start=False, stop=(jc == 3),
                            )
                    pg_sb = lp.tile([81, 2], F32, name="pg_sb", tag="pg_sb")
                    nc.vector.tensor_scalar_add(pg_sb[:], pg_ps[:], sb["bias_pg"][:, 0:1])
                    vmel = out_mel[0, 0, t - 1 : t].copy()
                    vmel.ap = mybir.VecI64Pair([[64, 80], [80 * 64, 2]])
                    nc.sync.dma_start(vmel, pg_sb[0:80, 0:2])
                    vgate = out_gate[0, t - 1 : t].copy()
                    vgate.ap = mybir.VecI64Pair([[64, 2]])
                    nc.sync.dma_start(vgate, pg_sb[80:81, 0:2])

                # ============ dec LSTM ============
                gd = pgd.tile([128, 64], F32, name="gd", tag="gd")
                for g in range(4):
                    for j in range(8):  # ah part
                        nc.tensor.matmul(
                            gd[:, g * 16 : g * 16 + 16],
                            sb["w1d"][:, (j * 4 + g) * 128 : (j * 4 + g + 1) * 128],
                            hd_full[:, j * 32 : j * 32 + 16],
                            start=(j == 0), stop=False,
                        )
                    for j2 in range(8):  # dh(t-1) part
                        nc.tensor.matmul(
                            gd[:, g * 16 : g * 16 + 16],
                            sb["w2d"][:, (j2 * 4 + g) * 128 : (j2 * 4 + g + 1) * 128],
                            hd_full[:, j2 * 32 + 16 : j2 * 32 + 32],
                            start=False, stop=False,
                        )
                    for jc in range(4):  # ctx(t) part
                        nc.tensor.matmul(
                            gd[:, g * 16 : g * 16 + 16],
                            sb["w1d"][:, ((8 + jc) * 4 + g) * 128 : ((8 + jc) * 4 + g + 1) * 128],
                            ctx_kt(ctxT_full, jc),
                            start=False, stop=(jc == 3),
                        )
                hdT = lp.tile([128, 16], BF16, name="hdT", tag="hdT")
                lstm_elem(gd, sb["b_d"], cd, hdT)
                tr_dh = ptr.tile([16, 128], BF16, name="tr_dh", tag="tr")
                nc.tensor.transpose(tr_dh[:], hdT[:], sb["ident"][:])
                dh_tr = lp.tile([16, 128], BF16, name="dh_tr")
                nc.vector.tensor_copy(dh_tr[:], tr_dh[:])

            # ============ epilogue: gather dh(T-1), final proj ============
            agf_in = dr.tile([16, 128], BF16, name="agf_in", tag="agf_in")
            agf_out = dr.tile([NCORES, 16, 128], BF16, name="agf_out", tag="agf_out",
                              addr_space="Shared")
            nc.sync.dma_start(agf_in[:], dh_tr[:])
            nc.gpsimd.collective_compute(
                "AllGather", ALU.bypass, replica_groups=RG,
                ins=[agf_in[:].opt()], outs=[agf_out[:].opt()],
            )
            dhT_fin = lp.tile([128, 128], BF16, name="dhT_full")
            nc.sync.dma_start_transpose(
                dhT_fin[:], agf_out[:].rearrange("j b p -> (j b) p"))
            pg_ps = pmB.tile([81, 2], F32, name="psB", tag="pmB")
            for bl in range(BLOC):
                for j in range(8):
                    nc.tensor.matmul(
                        pg_ps[:, bl : bl + 1], sb["wpg"][:, j * 81 : (j + 1) * 81],
                        dhT_fin[:, bass.ds(j * 16 + r2 + bl, 1)],
                        start=(j == 0), stop=False,
                    )
                for jc in range(4):
                    nc.tensor.matmul(
                        pg_ps[:, bl : bl + 1], sb["wpg"][:, (8 + jc) * 81 : (9 + jc) * 81],
                        ctxT_full[:, bass.ds((r2 + bl) * 4 + jc, 1)],
                        start=False, stop=(jc == 3),
                    )
            pg_sb = lp.tile([81, 2], F32, name="pg_sb", tag="pg_sb")
            nc.vector.tensor_scalar_add(pg_sb[:], pg_ps[:], sb["bias_pg"][:, 0:1])
            vmel = out_mel[0, 0, t_dec - 1 : t_dec].copy()
            vmel.ap = mybir.VecI64Pair([[64, 80], [80 * 64, 2]])
            nc.sync.dma_start(vmel, pg_sb[0:80, 0:2])
            vgate = out_gate[0, t_dec - 1 : t_dec].copy()
            vgate.ap = mybir.VecI64Pair([[64, 2]])
            nc.sync.dma_start(vgate, pg_sb[80:81, 0:2])

    nc.compile()
    return nc


def kernel(**inputs):
    global _COMPILED
    if _COMPILED is None:
        _COMPILED = build()
    nc = _COMPILED
    in_maps = prep_inputs(inputs)
    res = bass_utils.run_bass_kernel_spmd(nc, in_maps, core_ids=list(range(NCORES)))
    mel = np.concatenate([res.results[k]["out_mel"] for k in range(NCORES)], axis=0)
    gate = np.concatenate([res.results[k]["out_gate"] for k in range(NCORES)], axis=0)
    align = np.concatenate([res.results[k]["out_align"] for k in range(NCORES)], axis=0)
    return mel.astype(np.float32), gate.astype(np.float32), align.astype(np.float32)
